# revision 52
# baseline (speedup 1.0000x reference)
"""Dual-key additive attention (nn_Attention_58059367908033) on 8 trn2 NeuronCores.

Reference computation (per batch b, head h, n = 64*64 = 4096 positions, d = 128,
scale = d**-0.5):
    q  = Wq_h  @ fmap[b]          # [d, n]
    k1 = Wk1_h @ fmap[b]          # [d, n]
    v  = Wv_h  @ fmap[b]          # [d, n]
    k2 = Wk2_h @ x[b]             # [d, n]
    sim  = (scale * q)^T (k1+k2)  # [n, n]
    attn = softmax(sim, axis=-1)
    out[b, h*d:(h+1)*d] = (attn @ v^T)^T

Sharding: 8 cores = (b in 2) x (h in 2) x (key-half kh in 2).  Each core
computes unnormalized flash-attention partials over its 2048-key slice:
    U[d, q] = sum_k exp(scale*sim[k, q]) * vT[k, d]
    D[1, q] = sum_k exp(scale*sim[k, q])
and the host adds the two key-half partials and divides (exact softmax).

Schedule (v3): the Activation engine's exp stream (64 tiles x ~1.07us) is the
main-phase bottleneck, so the schedule keeps it saturated: attention runs in
3 query groups (qc {0,1,2}, {3,4,5}, {6,7}) whose U accumulate directly in
PSUM banks; ksum chunk builds are emitted in small pieces between attention
units so the PE never monopolizes more than the exp backlog; sim+exp for
later groups is precomputed during earlier groups' PV phases.  Host rotates
fmap columns per-core (key half first) so no separate fmap_k shipment is
needed; all DMA layouts are dense per-partition and scheduled by need-time
across the sync/scalar/gpsimd queues.  Denominators: exp tiles are summed on
DVE (slot 0) and GPSIMD (slot 1) into per-qc accumulators, merged and reduced
across partitions with one ones-matmul per query chunk.  Outputs stream out
per-qc as bf16.
"""

from collections import deque

import ml_dtypes
import numpy as np

BF16_NP = ml_dtypes.bfloat16

import concourse.bass as bass
import concourse.mybir as mybir
import concourse.tile as tile
from concourse import bacc
from concourse.bass_utils import run_bass_kernel_spmd

HEADS = 2
D = 128          # dim head
C1 = 256         # fmap channels
C2 = 2048        # x channels
N = 4096         # spatial positions = queries; keys per core = 2048
KSL = 2048
SCALE = float(D) ** -0.5

F32 = mybir.dt.float32
BF16 = mybir.dt.bfloat16

KC = 4           # key chunks per core (512 keys each)
QW = 512         # query-chunk width
QC = 8           # query chunks

_COMPILED = {}


def _build_program(warm1=26, warm2=6):
    nc = bacc.Bacc("TRN2", target_bir_lowering=False, debug=False, num_devices=8)

    # ---- DRAM parameters (per-core data, same program on all 8 cores) ----
    # All host-pretransposed to dense [128, cols...] layouts.
    # d_w slots: wq=0:2, wk1=2:4, wv=4:6, wk2=6:22  (each [128c, 128d])
    d_w = nc.dram_tensor("w", [128, 22, 128], BF16, kind="ExternalInput").ap()
    d_fmap = nc.dram_tensor("fmap", [128, 2, N], BF16, kind="ExternalInput").ap()
    d_xs = nc.dram_tensor("xs", [128, 128, 256], BF16, kind="ExternalInput").ap()
    d_outU = nc.dram_tensor("outU", [128, N], BF16, kind="ExternalOutput").ap()
    d_den = nc.dram_tensor("denom", [1, N], BF16, kind="ExternalOutput").ap()

    with tile.TileContext(nc) as tc:
        with (
            tc.tile_pool(name="wts", bufs=1) as wts,
            tc.tile_pool(name="fm", bufs=1) as fm,
            tc.tile_pool(name="xs", bufs=6) as xsp,
            tc.tile_pool(name="big", bufs=1) as big,
            tc.tile_pool(name="etA", bufs=6) as etA,
            tc.tile_pool(name="etB", bufs=24) as etB,
            tc.tile_pool(name="etC", bufs=16) as etC,
            tc.tile_pool(name="stg", bufs=2) as stg,
            tc.tile_pool(name="ps_u", bufs=3, space="PSUM") as ps_u,
            tc.tile_pool(name="ps_s", bufs=2, space="PSUM") as ps_s,
            tc.tile_pool(name="ps_x", bufs=1, space="PSUM") as ps_x,
        ):
            # ---- input DMAs, scheduled by need-time ----
            # Queue start order: sync (~8us) < scalar (~10us) < gpsimd (~12us).
            # need: small w + fmap key half ~13us (v/q/k1); wk2+x0 ~18us
            # (ksum0); x1 ~30us; fmap query half ~28us; x2 ~40us; x3 ~50us.
            w = wts.tile([128, 22, 128], BF16, tag="w")
            fmap = fm.tile([128, 2, N], BF16, tag="fmap")
            x_tiles = [xsp.tile([128, 16, 256], BF16, tag="x", name=f"x{i}")
                       for i in range(6)]

            def load_xp(p, eng):
                eng.dma_start(x_tiles[p % 6][:], d_xs[:, p * 16:(p + 1) * 16, :])

            def load_fk(c0, c1, eng):
                eng.dma_start(fmap[:, :, c0:c1], d_fmap[:, :, c0:c1])

            ones = wts.tile([128, 1], BF16, tag="ones")
            nc.vector.memset(ones[:], 1.0)
            warm = wts.tile([128, 512], BF16, tag="warm")
            nc.vector.memset(warm[:], 0.0)

            # DMA issue instructions block their issuing engine once the
            # hardware queue is deep, so the act engine (scalar) gets only
            # the 3 earliest issues; everything else rides sync / gpsimd,
            # whose streams have no compute.  Everything is need-ordered:
            # the first two x key-pairs and fmap cols 0:1024 unlock the
            # exp stream at ~16us; later pairs/pieces follow.
            nc.sync.dma_start(w[:, 0:6, :], d_w[:, 0:6, :])
            load_fk(0, 512, nc.scalar)
            nc.gpsimd.dma_start(w[:, 6:22, :], d_w[:, 6:22, :])
            load_xp(0, nc.sync)
            load_xp(1, nc.scalar)
            load_fk(512, 1024, nc.scalar)
            load_fk(1024, 1536, nc.sync)
            load_xp(2, nc.sync)
            load_xp(3, nc.gpsimd)
            load_fk(1536, 2048, nc.gpsimd)
            load_xp(4, nc.sync)
            load_xp(5, nc.gpsimd)
            load_fk(KSL, KSL + 512, nc.sync)
            load_fk(KSL + 512, KSL + 1024, nc.gpsimd)
            nc.sync.dma_start(fmap[:, 0, KSL + 1024:N],
                              d_fmap[:, 0, KSL + 1024:N])
            nc.gpsimd.dma_start(fmap[:, 1, KSL + 1024:N],
                                d_fmap[:, 1, KSL + 1024:N])

            # ---- persistent SBUF tensors ----
            q_sb = big.tile([128, N], BF16, tag="q")
            ksum = big.tile([128, KSL], BF16, tag="ksum")
            vT = big.tile([128, 16, D], BF16, tag="vT")
            acc = big.tile([128, QC, 2, QW], BF16, tag="acc")
            den_stage = big.tile([1, N], BF16, tag="den")

            # ---- HAM warm-up: dummy matmuls keep the PE active during the
            # DMA-bound prologue so the clock gate opens early.
            def warmup(n):
                for _ in range(n):
                    wps = ps_x.tile([128, QW], F32, tag="x", name="pswarm")
                    nc.tensor.matmul(wps[:], warm[:, :128], warm[:],
                                     start=True, stop=True)

            warmup(warm1)

            # ---- vT tiles [k=128, d] in groups of 4 per PSUM bank; group 0
            # is needed by the first PVs, groups 1-3 weave into step 0 ----
            def build_v(g):
                psv = ps_x.tile([128, 4, D], F32, tag="x", name="psv")
                for i in range(4):
                    kk = g * 4 + i
                    ksl = slice(kk * 128, (kk + 1) * 128)
                    nc.tensor.matmul(psv[:, i, :], fmap[:, 0, ksl], w[:, 4, :],
                                     start=True, stop=False)
                    nc.tensor.matmul(psv[:, i, :], fmap[:, 1, ksl], w[:, 5, :],
                                     start=False, stop=True)
                nc.scalar.copy(vT[:, g * 4:(g + 1) * 4, :], psv[:])

            build_v(0)

            # ---- q projection chunks (allocated from the sim psum pool so
            # they never alias a live ksum-build accumulator) ----
            def build_q(nch):
                psq = ps_s.tile([128, 2, QW], F32, tag="s", name="psq")
                sl = slice(nch * QW, (nch + 1) * QW)
                nc.tensor.matmul(psq[:, 0, :], w[:, 0, :], fmap[:, 0, sl],
                                 start=True, stop=False)
                nc.tensor.matmul(psq[:, 0, :], w[:, 1, :], fmap[:, 1, sl],
                                 start=False, stop=True)
                nc.vector.tensor_copy(q_sb[:, sl], psq[:, 0, :])

            build_q(0)
            build_q(1)
            warmup(warm2)

            # ---- ksum build per 256-key pair: k1 + k2 over the pair's
            # columns; one pair is the granularity at which sims unlock ----
            def build_pair(p):
                kps = ps_x.tile([128, QW], F32, tag="x", name="kps")
                kv = kps[:, 0:256]
                sl = slice(p * 256, (p + 1) * 256)
                nc.tensor.matmul(kv, w[:, 2, :], fmap[:, 0, sl],
                                 start=True, stop=False)
                nc.tensor.matmul(kv, w[:, 3, :], fmap[:, 1, sl],
                                 start=False, stop=False)
                xt = x_tiles[p % 6]
                for t in range(16):
                    nc.tensor.matmul(kv, w[:, 6 + t, :], xt[:, t, :],
                                     start=False, stop=(t == 15))
                nc.vector.tensor_copy(ksum[:, sl], kv)

            build_pair(0)
            load_xp(6, nc.sync)
            build_pair(1)
            load_xp(7, nc.gpsimd)

            # ---- attention machinery ----
            acc_first = [True] * QC
            pend = deque()          # deferred PE work (PV closures)

            def pop_pend():
                if pend:
                    pend.popleft()()

            def emit_sim_tile(kc, qc, half, pool):
                """sim matmuls + exp + denominator accumulate for key tiles
                (kc*4+2*half, +1) x query chunk qc.  Returns the exp tile."""
                sps = ps_s.tile([128, 2, QW], F32, tag="s", name="sps")
                qsl = slice(qc * QW, (qc + 1) * QW)
                for j in range(2):
                    kk = kc * 4 + 2 * half + j
                    nc.tensor.matmul(sps[:, j, :],
                                     ksum[:, kk * 128:(kk + 1) * 128],
                                     q_sb[:, qsl], start=True, stop=True)
                et = pool.tile([128, 2, QW], BF16, tag="et", name="et")
                nc.scalar.activation(et[:], sps[:],
                                     mybir.ActivationFunctionType.Exp,
                                     scale=SCALE)
                if acc_first[qc]:
                    nc.vector.tensor_copy(acc[:, qc], et[:])
                    acc_first[qc] = False
                else:
                    nc.vector.tensor_add(acc[:, qc], acc[:, qc], et[:])
                return et

            def make_pv(kc, qc, ets, U):
                def _pv():
                    for half in range(2):
                        for j in range(2):
                            kk = kc * 4 + 2 * half + j
                            nc.tensor.matmul(U[:], vT[:, kk, :],
                                             ets[half][:, j, :],
                                             start=(kc == 0 and kk == 0),
                                             stop=(kc == 3 and kk == 15))
                return _pv

            den_done = set()

            def emit_den(qc, tail=False):
                """denominator reduce for qc: needs all 16 of qc's exp tiles
                accumulated, but not the U bank -- usable as PE filler."""
                qsl = slice(qc * QW, (qc + 1) * QW)
                dsum = stg.tile([128, QW], BF16, tag="dsum", name="dsum")
                nc.vector.tensor_add(dsum[:], acc[:, qc, 0, :], acc[:, qc, 1, :])
                dn = ps_x.tile([1, QW], F32, tag="x", name="dn")
                nc.tensor.matmul(dn[:], ones[:], dsum[:], start=True, stop=True)
                if tail:
                    nc.scalar.copy(den_stage[:, qsl], dn[:])
                else:
                    nc.vector.tensor_copy(den_stage[:, qsl], dn[:])
                den_done.add(qc)

            def drain(qc, U, tail=False):
                """output store for a finished qc.  Mid-kernel drains copy on
                DVE; tail drains use the act engine, which is idle by then."""
                if qc not in den_done:
                    emit_den(qc, tail)
                qsl = slice(qc * QW, (qc + 1) * QW)
                u_st = stg.tile([128, QW], BF16, tag="u_st", name="u_st")
                if tail:
                    nc.scalar.copy(u_st[:], U[:])
                else:
                    nc.vector.tensor_copy(u_st[:], U[:])
                nc.sync.dma_start(d_outU[:, qsl], u_st[:])

            # ---- phase A: qc group {0,1,2} full attention; ksum builds
            # (in pieces) and group-B sim/exp tiles woven in ----
            U_A = {qc: ps_u.tile([128, QW], F32, tag="u", name=f"ua{qc}")
                   for qc in (0, 1, 2)}
            etsB = {}

            def emit_A(kc, qc):
                e0 = emit_sim_tile(kc, qc, 0, etA)
                pop_pend()
                e1 = emit_sim_tile(kc, qc, 1, etA)
                pop_pend()
                pend.append(make_pv(kc, qc, (e0, e1), U_A[qc]))

            def emit_B(bkc, bqc):
                e0 = emit_sim_tile(bkc, bqc, 0, etB)
                pop_pend()
                e1 = emit_sim_tile(bkc, bqc, 1, etB)
                pop_pend()
                etsB[(bkc, bqc)] = (e0, e1)

            # Explicit per-step schedule: A=full unit, B=sim/exp only (PV in
            # phase B), P=next ksum chunk's build piece (piece 0 is the
            # x-independent k1 part), Q=q-chunk projection.  B units of chunk
            # kc' require ksum[kc'] (finished during step kc'-1) and balance
            # the act engine against the PE's build work.
            stepsA = [
                [("A", 0, 0), ("V", 1), ("A", 0, 1), "P", ("Q", 2),
                 ("A", 0, 2), "P", ("V", 2), ("Q", 3), ("B", 0, 3),
                 ("Q", 4), ("B", 0, 4), ("V", 3), ("Q", 5), ("B", 0, 5)],
                [("A", 1, 0), "P", ("A", 1, 1), "P", ("A", 1, 2),
                 ("B", 1, 3), "W", ("B", 1, 4), "W", ("B", 1, 5), "W"],
                [("A", 2, 0), "P", ("A", 2, 1), "P", ("A", 2, 2),
                 ("B", 2, 3), "W", ("B", 2, 4), "W", ("B", 2, 5), "W"],
                [("A", 3, 0), ("A", 3, 1), ("D", 0), ("A", 3, 2), ("D", 1),
                 ("B", 3, 3), ("D", 2), ("B", 3, 4), "W", ("B", 3, 5), "W"],
            ]
            for kc, step in enumerate(stepsA):
                pieces = ([lambda kc=kc: build_pair(2 * kc + 2),
                           lambda kc=kc: build_pair(2 * kc + 3)]
                          if kc + 1 < KC else [])
                for op in step:
                    if op == "P":
                        if pieces:
                            pieces.pop(0)()
                    elif op == "W":
                        warmup(2)
                    elif op[0] == "A":
                        emit_A(op[1], op[2])
                    elif op[0] == "B":
                        emit_B(op[1], op[2])
                    elif op[0] == "Q":
                        build_q(op[1])
                    elif op[0] == "V":
                        build_v(op[1])
                    elif op[0] == "D":
                        emit_den(op[1])
                while pieces:
                    pieces.pop(0)()
            while pend:
                pop_pend()

            # ---- phase B: PV group B from stored exp tiles; group-A drains,
            # q builds and group-C sim/exp spread between PV units so neither
            # the act engine nor the PSUM banks ever go cold ----
            drain(0, U_A[0])
            build_q(6)
            U_B = {}
            etsC = {}
            preC = [(kc, qc) for qc in (6, 7) for kc in range(KC)]
            ci = 0
            for qc in (3, 4, 5):
                U_B[qc] = ps_u.tile([128, QW], F32, tag="u", name=f"ub{qc}")
                for kc in range(KC):
                    make_pv(kc, qc, etsB[(kc, qc)], U_B[qc])()
                    # weave ~2 C sim tiles per 3 PV units, front-loaded
                    while ci * 3 < 2 * ((qc - 3) * 4 + kc + 1) + 2 and ci < 8:
                        ckc, cqc = preC[ci]
                        e0 = emit_sim_tile(ckc, cqc, 0, etC)
                        e1 = emit_sim_tile(ckc, cqc, 1, etC)
                        etsC[(ckc, cqc)] = (e0, e1)
                        ci += 1
                drain(qc, U_B[qc])
                if qc == 3:
                    drain(1, U_A[1])
                    build_q(7)
                elif qc == 4:
                    drain(2, U_A[2])

            # ---- phase C: PV group C, drained per qc (tail path) ----
            U_C = {qc: ps_u.tile([128, QW], F32, tag="u", name=f"uc{qc}")
                   for qc in (6, 7)}
            for qc in (6, 7):
                for kc in range(KC):
                    if (kc, qc) not in etsC:
                        e0 = emit_sim_tile(kc, qc, 0, etC)
                        e1 = emit_sim_tile(kc, qc, 1, etC)
                        etsC[(kc, qc)] = (e0, e1)
                    make_pv(kc, qc, etsC[(kc, qc)], U_C[qc])()
                drain(qc, U_C[qc], tail=True)
            nc.sync.dma_start(d_den[:], den_stage[:])

    nc.compile()
    return nc


def _prep_inputs(fmap, x, Wqkv, Wk2):
    """Host-side slicing/transposition: per-core input dicts.
    Core c = b*4 + h*2 + kh.  fmap columns are rotated so the core's
    key half is always cols 0..2047 (outputs are un-rotated in _combine)."""
    fmap = np.ascontiguousarray(fmap, dtype=np.float32)
    x = np.ascontiguousarray(x, dtype=np.float32)
    Wqkv = np.ascontiguousarray(Wqkv, dtype=np.float32)
    Wk2 = np.ascontiguousarray(Wk2, dtype=np.float32)

    in_maps = []
    for c in range(8):
        b, h, kh = c // 4, (c // 2) % 2, c % 2
        fb = fmap[b].reshape(C1, N)
        xb = x[b].reshape(C2, N)
        # rotate fmap columns: key half first
        fb_r = np.roll(fb, -kh * KSL, axis=1)
        # weights: [22, 128c_part, 128d] -> transpose to [128, 22, 128]
        w = np.empty((22, 128, 128), dtype=np.float32)
        wq = Wqkv[h * D:(h + 1) * D]              # [128, 256]
        wk1 = Wqkv[C1 + h * D:C1 + (h + 1) * D]
        wv = Wqkv[2 * C1 + h * D:2 * C1 + (h + 1) * D]
        wk2 = Wk2[h * D:(h + 1) * D]              # [128, 2048]
        w[0:2] = wq.T.reshape(2, 128, D)
        w[2:4] = wk1.T.reshape(2, 128, D)
        w[4:6] = wv.T.reshape(2, 128, D)
        w[6:22] = wk2.T.reshape(16, 128, D)
        # x key slice packed pair-major: [128 part, p*16+t, 256]
        xsl = xb[:, kh * KSL:(kh + 1) * KSL]      # [2048, 2048]
        xs = (xsl.reshape(16, 128, 8, 256)        # [t, part, pair, n]
                 .transpose(1, 2, 0, 3)           # [part, pair, t, n]
                 .reshape(128, 128, 256))
        in_maps.append({
            "w": np.ascontiguousarray(w.transpose(1, 0, 2)).astype(BF16_NP),
            "fmap": np.ascontiguousarray(
                fb_r.reshape(2, 128, N).transpose(1, 0, 2)).astype(BF16_NP),
            "xs": np.ascontiguousarray(xs).astype(BF16_NP),
        })
    return in_maps


def _combine(results):
    """Host epilogue: un-rotate, add key-half partials, normalize."""
    out = np.empty((2, HEADS * D, 64, 64), dtype=np.float32)
    for b in range(2):
        for h in range(2):
            c0 = b * 4 + h * 2
            U0 = results[c0]["outU"].astype(np.float32)
            D0 = results[c0]["denom"]
            U1 = np.roll(results[c0 + 1]["outU"].astype(np.float32), KSL, axis=1)
            D1 = np.roll(results[c0 + 1]["denom"], KSL, axis=1)
            out[b, h * D:(h + 1) * D] = ((U0 + U1) / (D0 + D1)).reshape(D, 64, 64)
    return out


def run_on_device(in_maps, trace=False, **kw):
    if "nc" not in _COMPILED:
        _COMPILED["nc"] = _build_program()
    return run_bass_kernel_spmd(_COMPILED["nc"], in_maps, list(range(8)),
                                trace=trace, **kw)


def kernel(fmap, x, Wqkv, Wk2):
    in_maps = _prep_inputs(fmap, x, Wqkv, Wk2)
    res = run_on_device(in_maps)
    return _combine(res.results)
